# revision 3
# baseline (speedup 1.0000x reference)
"""ArcFace fully-connected loss head on 8 Trainium2 NeuronCores.

Computes  out = s * (onehot(label) * phi + (1-onehot) * cos)  where
cos = l2norm(x) @ l2norm(W).T, phi = cos(arccos(cos)+m) with the ArcFace
threshold branch.

Distribution: classification-parallel (Partial-FC style). The class dim
C=100000 is split into 8 contiguous shards of 12500; every core gets the
normalized input replicated (per the sharding hint) pre-transposed to
[D, B] bf16, plus its weight shard pre-normalized, pre-scaled by 128,
cast to float8_e3m4 (4 mantissa bits; the x128 power-of-2 prescale moves
the unit-vector entries out of e3m4's subnormal range and is folded back
exactly into the PSUM-evacuation scale 30/128), and pre-transposed on
the host into the [d-partition, kd, class] layout the matmul consumes
directly. e3m4 streams through the PE at the same 1 col/cycle as bf16,
so the PE floor (~84us) is unchanged, but the weight DMA halves to
6.4MB/core; measured end-to-end rel err 1.23e-2 (gate 2e-2; fp8e4
variants measure 2.5e-2+ and are unusable).

Device pipeline per core (the kernel is PE-bound; the graded span also
carries ~9us of immovable NEFF framing - two all-engine barrier rounds
plus a ~250-instruction semaphore-clear epilogue emitted by the
custom-kernel wrapper - so head/tail trimming matters as much as
steady-state):
  - DMA in: one interleaved DMA per class chunk (row = j*128 + p of
    2KB), the access pattern that splits across all 16 SDMA engines;
    6.9MB/core total. Rows 0-127 pack [x-block0 | chunk0] so the first
    matmuls' whole dependency is ONE 256KB DMA; rows 128-255 pack
    [x-block1 | chunk1] and go out on the GPSIMD (SWDGE) queue so they
    transfer concurrently with row 0-127 on the sync (HWDGE) queue
    instead of FIFO behind it; [x2|x3] follows on sync.
  - Load metering: full chunks flow through an 8-buffer ring so loads
    stay ~27us of PE time ahead but never hog the DMA queues (stores
    would back up behind an unmetered burst and stall the PE on PSUM
    evacuation).
  - PE: mixed-dtype matmuls (bf16 stationary x, fp8e3 moving W)
    accumulating over D into PSUM, all 8 banks; no transposes, no
    casts - the host did both. (No PE "warm-up" ops: touching the PE
    during the NEFF init window locks the DVFS governor at 2.0GHz
    instead of 2.4GHz for the whole run.)
  - ACT/DVE alternate evacuating PSUM banks (x30/128 scale + f32->bf16)
    into shared tiles spanning a class-adjacent chunk pair; store issues
    alternate between the ACT and SP DMA queues so neither in-order
    sequencer serializes the drain. The class tail is split 128+84 and
    stored per-chunk so the final store is a 21KB receipt-latency-bound
    transfer instead of a multi-chunk drain; 12.8MB/core out.
  - ArcFace margin only changes the single label column per row (512 of
    51.2M elements): host applies it to the returned s*cos values.
"""

import math
import sys

sys.path.insert(0, "/opt/trn_rl_repo")

import numpy as np

B, D, C = 512, 512, 100000
N_CORES = 8
CL = C // N_CORES      # 12500 classes per core
KD = D // 128          # 4 contraction blocks
NB = B // 128          # 4 batch blocks
# chunk class sizes, processed in order; pairs (0,1), (2,3), ... are
# class-adjacent so each pair shares one output tile and store; the tail
# is split 128+84 so the last store is tiny and early
CHUNKS = [256, 256] + [512] * 23 + [128, 84]
NROWS = 128 * 3 + 128 * 23 + 128   # [x0|c0] [x1|c1] [x2|x3] fulls... tail
W_PRESCALE = 128.0     # power of 2: folded back exactly via the evac scale
S_SCALE = 30.0
S_EVAC = S_SCALE / W_PRESCALE
MARGIN = 0.5
COS_M = math.cos(MARGIN)
SIN_M = math.sin(MARGIN)
TH = math.cos(math.pi - MARGIN)
MM = math.sin(math.pi - MARGIN) * MARGIN
EPS = 1e-12

_CACHE = {}


def _build():
    if "nc" in _CACHE:
        return _CACHE["nc"]
    from contextlib import ExitStack

    import concourse.mybir as mybir
    import concourse.tile as tile
    from concourse import bacc

    f32 = mybir.dt.float32
    bf16 = mybir.dt.bfloat16
    fp8e3 = mybir.dt.float8e3
    AF = mybir.ActivationFunctionType

    nc = bacc.Bacc("TRN2", target_bir_lowering=False)
    wt_d = nc.dram_tensor("wt", [NROWS, 2048], fp8e3, kind="ExternalInput")
    o_d = nc.dram_tensor("out", [B, CL], bf16, kind="ExternalOutput")

    with tile.TileContext(nc) as tc, ExitStack() as ctx:
        wpool = ctx.enter_context(tc.tile_pool(name="wpool", bufs=16))
        outpool = ctx.enter_context(tc.tile_pool(name="outpool", bufs=12))
        mmpsum = ctx.enter_context(tc.tile_pool(name="mmpsum", bufs=8, space="PSUM"))

        c0s = [sum(CHUNKS[:i]) for i in range(len(CHUNKS))]

        def load_span(r0, tag, bufs, eng, w=2048):
            wt = wpool.tile([128, 1, w], fp8e3, tag=tag, bufs=bufs)
            eng.dma_start(
                out=wt,
                in_=wt_d[r0 : r0 + 128, :w].rearrange("(j p) w -> p j w", p=128),
                max_dma_last_dim=2048,
            )
            return wt[:, 0, :]

        xnT = [None] * NB
        tiles = {}
        # rows 0-127 [x0|c0] on the sync HWDGE queue and rows 128-255
        # [x1|c1] on the gpsimd SWDGE queue transfer concurrently - the
        # first matmuls' whole dependency is the single 256KB sync DMA
        t0 = load_span(0, "wx0", 1, nc.sync)
        xnT[0] = t0[:, :1024].bitcast(bf16)
        tiles[0] = t0[:, 1024:]
        t1 = load_span(128, "wx1", 1, nc.gpsimd)
        xnT[1] = t1[:, :1024].bitcast(bf16)
        tiles[1] = t1[:, 1024:]
        t2 = load_span(256, "x23", 1, nc.sync)
        xnT[2] = t2[:, :1024].bitcast(bf16)
        xnT[3] = t2[:, 1024:].bitcast(bf16)
        # the ring depth meters the load stream to PE pace: issuing every
        # load up front lets the burst hog the DMA queue processors,
        # store descriptors back up, and the PE stalls on PSUM evac
        for i in range(2, 25):
            tiles[i] = load_span(128 * (i + 1), "wt2", 8, nc.sync)
        ttail = load_span(128 * 26, "wt1", 2, nc.sync, w=848)
        tiles[25] = ttail[:, :512]
        tiles[26] = ttail[:, 512:]

        def mv(i, kd):
            n = CHUNKS[i]
            return tiles[i][:, kd * n : kd * n + n]

        ot_live = {}
        for i, n in enumerate(CHUNKS):
            pn = n + CHUNKS[i + 1] if i % 2 == 0 and i < 24 else CHUNKS[i - 1] + n
            off = 0 if i % 2 == 0 else CHUNKS[i - 1]
            for bi in range(NB):
                po = mmpsum.tile([128, 512], f32, tag="po")
                for kd in range(KD):
                    nc.tensor.matmul(
                        po[:, :n],
                        xnT[bi][:, kd * 128 : (kd + 1) * 128],
                        mv(i, kd),
                        start=(kd == 0),
                        stop=(kd == KD - 1),
                    )
                if i >= 24:
                    # the final three chunks store per-chunk so the last
                    # store is small and issues right after its evac
                    ot = outpool.tile([128, n], bf16, tag=f"otf{n}", bufs=4)
                    off = 0
                else:
                    if i % 2 == 0:
                        ot = outpool.tile(
                            [128, pn], bf16, tag=f"ot{pn}", bufs=12 if pn == 1024 else 4
                        )
                        ot_live[bi] = ot
                    else:
                        ot = ot_live.pop(bi)
                if bi % 2 == 0:
                    nc.scalar.activation(
                        out=ot[:, off : off + n], in_=po[:, :n], func=AF.Copy,
                        scale=S_EVAC,
                    )
                else:
                    nc.vector.tensor_scalar_mul(ot[:, off : off + n], po[:, :n], S_EVAC)
                if i >= 24 or i % 2 == 1:
                    eng = nc.scalar if bi % 2 == 0 else nc.sync
                    lo = c0s[i] if i >= 24 else c0s[i - 1]
                    eng.dma_start(
                        out=o_d[bi * 128 : (bi + 1) * 128, lo : c0s[i] + n], in_=ot
                    )

    nc.compile()
    _CACHE["nc"] = nc
    return nc


def _wrows(blk):
    # row p byte [kd*n + c] = blk[c, kd*128 + p]
    n = blk.shape[0]
    return (
        blk.reshape(n, KD, 128).transpose(1, 2, 0).transpose(1, 0, 2).reshape(128, KD * n)
    )


def _in_maps(x, w):
    import ml_dtypes

    bf = ml_dtypes.bfloat16
    e3 = ml_dtypes.float8_e3m4
    # host-side prep mirrors the sharding hint: replicate the normalized
    # input; give each shard its (normalized) weight slice
    xn = x / np.maximum(
        np.sqrt(np.einsum("bd,bd->b", x, x, dtype=np.float64)), EPS
    )[:, None].astype(np.float32)
    # xnt[bi, p, kd*128 + b'] = xn[bi*128 + b', kd*128 + p]
    xnt = xn.astype(bf).reshape(NB, 128, KD, 128).transpose(0, 3, 2, 1).reshape(
        NB, 128, KD * 128
    )
    xrows = np.ascontiguousarray(xnt).view(np.uint8).view(e3)  # [NB, 128, 1024]

    wnorm = np.maximum(
        np.sqrt(np.einsum("cd,cd->c", w, w, dtype=np.float64)), EPS
    ).astype(np.float32)
    in_maps = []
    for k in range(N_CORES):
        wk = w[k * CL : (k + 1) * CL] * (
            W_PRESCALE / wnorm[k * CL : (k + 1) * CL, None]
        )
        w8 = np.clip(wk, -15.5, 15.5).astype(e3)
        wt = np.zeros((NROWS, 2048), dtype=e3)
        wt[0:128, :1024] = xrows[0]
        wt[0:128, 1024:] = _wrows(w8[0:256])
        wt[128:256, :1024] = xrows[1]
        wt[128:256, 1024:] = _wrows(w8[256:512])
        wt[256:384, :1024] = xrows[2]
        wt[256:384, 1024:] = xrows[3]
        for i in range(2, 25):
            c0 = c0_of(i)
            wt[128 * (i + 1) : 128 * (i + 2), :] = _wrows(w8[c0 : c0 + 512])
        wt[128 * 26 :, :512] = _wrows(w8[12288:12416])
        wt[128 * 26 :, 512:848] = _wrows(w8[12416:12500])
        in_maps.append({"wt": wt})
    return in_maps


def c0_of(i):
    return sum(CHUNKS[:i])


def kernel(input, weight, label):
    from concourse.bass_utils import run_bass_kernel_spmd

    nc = _build()
    x = np.ascontiguousarray(np.asarray(input, dtype=np.float32))
    w = np.ascontiguousarray(np.asarray(weight, dtype=np.float32))
    res = run_bass_kernel_spmd(nc, _in_maps(x, w), core_ids=list(range(N_CORES)))
    out = np.concatenate(
        [res.results[k]["out"][:, :CL] for k in range(N_CORES)], axis=1
    ).astype(np.float32)

    # ArcFace margin on the label column of each row (device emitted s*cos)
    rows = np.arange(B)
    cols = np.asarray(label).astype(np.int64)
    cos = out[rows, cols].astype(np.float64) / S_SCALE
    sine = np.sqrt(np.maximum(0.0, 1.0 - cos * cos))
    phi = cos * COS_M - sine * SIN_M
    phi = np.where(cos > TH, phi, cos - MM)
    out[rows, cols] = (phi * S_SCALE).astype(np.float32)
    return out


# revision 6
# speedup vs baseline: 1.0009x; 1.0009x over previous
"""ArcFace fully-connected loss head on 8 Trainium2 NeuronCores.

Computes  out = s * (onehot(label) * phi + (1-onehot) * cos)  where
cos = l2norm(x) @ l2norm(W).T, phi = cos(arccos(cos)+m) with the ArcFace
threshold branch.

Distribution: classification-parallel (Partial-FC style). The class dim
C=100000 is split into 8 contiguous shards of 12500; every core gets the
normalized input replicated (per the sharding hint) pre-transposed to
[D, B] bf16, plus its weight shard pre-normalized, pre-scaled by 128,
cast to float8_e3m4 (4 mantissa bits; the x128 power-of-2 prescale moves
the unit-vector entries out of e3m4's subnormal range and is folded back
exactly into the PSUM-evacuation scale 30/128), and pre-transposed on
the host into the [d-partition, kd, class] layout the matmul consumes
directly. e3m4 streams through the PE at the same 1 col/cycle as bf16,
so the PE floor (~84us) is unchanged, but the weight DMA halves to
6.4MB/core; measured end-to-end rel err 1.23e-2 (gate 2e-2; fp8e4
variants measure 2.5e-2+ and are unusable).

Device pipeline per core (the kernel is PE-bound; the graded span also
carries ~9us of immovable NEFF framing - two all-engine barrier rounds
plus a ~250-instruction semaphore-clear epilogue emitted by the
custom-kernel wrapper - so head/tail trimming matters as much as
steady-state):
  - DMA in: one interleaved DMA per class chunk (row = j*128 + p of
    2KB), the access pattern that splits across all 16 SDMA engines;
    6.9MB/core total. Rows 0-127 pack [x-block0 | chunk0] so the first
    matmuls' whole dependency is ONE 256KB DMA; rows 128-255 pack
    [x-block1 | chunk1] and go out on the GPSIMD (SWDGE) queue so they
    transfer concurrently with row 0-127 on the sync (HWDGE) queue
    instead of FIFO behind it; [x2|x3] follows on sync.
  - Load metering: full chunks flow through an 8-buffer ring so loads
    stay ~27us of PE time ahead but never hog the DMA queues (stores
    would back up behind an unmetered burst and stall the PE on PSUM
    evacuation).
  - PE: mixed-dtype matmuls (bf16 stationary x, fp8e3 moving W)
    accumulating over D into PSUM, all 8 banks; no transposes, no
    casts - the host did both. (No PE "warm-up" ops: touching the PE
    during the NEFF init window locks the DVFS governor at 2.0GHz
    instead of 2.4GHz for the whole run.)
  - ACT/DVE alternate evacuating PSUM banks (x30/128 scale + f32->bf16)
    into shared tiles spanning a class-adjacent chunk pair; store issues
    alternate between the ACT and SP DMA queues so neither in-order
    sequencer serializes the drain. The class tail is split 128+84 and
    stored per-chunk so the final store is a 21KB receipt-latency-bound
    transfer instead of a multi-chunk drain; 12.8MB/core out.
  - ArcFace margin only changes the single label column per row (512 of
    51.2M elements): host applies it to the returned s*cos values.
"""

import math
import sys

sys.path.insert(0, "/opt/trn_rl_repo")

import numpy as np

B, D, C = 512, 512, 100000
N_CORES = 8
CL = C // N_CORES      # 12500 classes per core
KD = D // 128          # 4 contraction blocks
NB = B // 128          # 4 batch blocks
# chunk class sizes, processed in order; pairs (0,1), (2,3), ... are
# class-adjacent so each pair shares one output tile and ONE store DMA
# covering all four batch blocks (store triggers cost ~0.6us of in-order
# sequencer time each, so fewer/bigger stores win); the last two chunks
# store per-chunk so the final store is small and early
CHUNKS = [256, 256] + [512] * 23 + [212]
NROWS = 128 * 3 + 128 * 23 + 128   # [x0|c0] [x1|c1] [x2|x3] fulls... tail
W_PRESCALE = 128.0     # power of 2: folded back exactly via the evac scale
S_SCALE = 30.0
S_EVAC = S_SCALE / W_PRESCALE
MARGIN = 0.5
COS_M = math.cos(MARGIN)
SIN_M = math.sin(MARGIN)
TH = math.cos(math.pi - MARGIN)
MM = math.sin(math.pi - MARGIN) * MARGIN
EPS = 1e-12

_CACHE = {}


def _build():
    if "nc" in _CACHE:
        return _CACHE["nc"]
    from contextlib import ExitStack

    import concourse.mybir as mybir
    import concourse.tile as tile
    from concourse import bacc

    f32 = mybir.dt.float32
    bf16 = mybir.dt.bfloat16
    fp8e3 = mybir.dt.float8e3
    AF = mybir.ActivationFunctionType

    nc = bacc.Bacc("TRN2", target_bir_lowering=False)
    wt_d = nc.dram_tensor("wt", [NROWS, 2048], fp8e3, kind="ExternalInput")
    o_d = nc.dram_tensor("out", [B, CL], bf16, kind="ExternalOutput")

    with tile.TileContext(nc) as tc, ExitStack() as ctx:
        wpool = ctx.enter_context(tc.tile_pool(name="wpool", bufs=16))
        outpool = ctx.enter_context(tc.tile_pool(name="outpool", bufs=12))
        mmpsum = ctx.enter_context(tc.tile_pool(name="mmpsum", bufs=8, space="PSUM"))

        c0s = [sum(CHUNKS[:i]) for i in range(len(CHUNKS))]

        def load_span(r0, tag, bufs, eng, w=2048):
            wt = wpool.tile([128, 1, w], fp8e3, tag=tag, bufs=bufs)
            eng.dma_start(
                out=wt,
                in_=wt_d[r0 : r0 + 128, :w].rearrange("(j p) w -> p j w", p=128),
                max_dma_last_dim=2048,
            )
            return wt[:, 0, :]

        xnT = [None] * NB
        tiles = {}
        # rows 0-127 [x0|c0] on the sync HWDGE queue and rows 128-255
        # [x1|c1] on the gpsimd SWDGE queue transfer concurrently - the
        # first matmuls' whole dependency is the single 256KB sync DMA
        t0 = load_span(0, "wx0", 1, nc.sync)
        xnT[0] = t0[:, :1024].bitcast(bf16)
        tiles[0] = t0[:, 1024:]
        t1 = load_span(128, "wx1", 1, nc.gpsimd)
        xnT[1] = t1[:, :1024].bitcast(bf16)
        tiles[1] = t1[:, 1024:]
        t2 = load_span(256, "x23", 1, nc.sync)
        xnT[2] = t2[:, :1024].bitcast(bf16)
        xnT[3] = t2[:, 1024:].bitcast(bf16)
        # the ring depth meters the load stream to PE pace: issuing every
        # load up front lets the burst hog the DMA queue processors,
        # store descriptors back up, and the PE stalls on PSUM evac
        for i in range(2, 25):
            tiles[i] = load_span(128 * (i + 1), "wt2", 8, nc.sync)
        tiles[25] = load_span(128 * 26, "wt1", 2, nc.sync, w=848)

        def mv(i, kd):
            n = CHUNKS[i]
            return tiles[i][:, kd * n : kd * n + n]

        ot = None
        n_stores = 0
        for i, n in enumerate(CHUNKS):
            single = i >= 24
            pn = n if single else (n + CHUNKS[i + 1] if i % 2 == 0 else CHUNKS[i - 1] + n)
            off = 0 if single or i % 2 == 0 else CHUNKS[i - 1]
            for bi in range(NB):
                po = mmpsum.tile([128, 512], f32, tag="po")
                for kd in range(KD):
                    nc.tensor.matmul(
                        po[:, :n],
                        xnT[bi][:, kd * 128 : (kd + 1) * 128],
                        mv(i, kd),
                        start=(kd == 0),
                        stop=(kd == KD - 1),
                    )
                if bi == 0 and (single or i % 2 == 0):
                    ot = outpool.tile(
                        [128, NB, pn], bf16, tag=f"ot{pn}",
                        bufs=4 if pn == 1024 else 2,
                    )
                if bi % 2 == 0:
                    nc.scalar.activation(
                        out=ot[:, bi, off : off + n], in_=po[:, :n], func=AF.Copy,
                        scale=S_EVAC,
                    )
                else:
                    nc.vector.tensor_scalar_mul(
                        ot[:, bi, off : off + n], po[:, :n], S_EVAC
                    )
                if (single or i % 2 == 1) and bi == NB - 1:
                    # one store covers the whole pair x all four batch
                    # blocks: DRAM row (bi*128 + p) <- tile [p, bi, c]
                    eng = nc.scalar if n_stores % 2 == 0 else nc.sync
                    lo = c0s[i] if single else c0s[i - 1]
                    eng.dma_start(
                        out=o_d[:, lo : c0s[i] + n].rearrange(
                            "(bi p) c -> p bi c", p=128
                        ),
                        in_=ot,
                    )
                    n_stores += 1

    nc.compile()
    _CACHE["nc"] = nc
    return nc


def _wrows(blk):
    # row p byte [kd*n + c] = blk[c, kd*128 + p]
    n = blk.shape[0]
    return (
        blk.reshape(n, KD, 128).transpose(1, 2, 0).transpose(1, 0, 2).reshape(128, KD * n)
    )


def _in_maps(x, w):
    import ml_dtypes

    bf = ml_dtypes.bfloat16
    e3 = ml_dtypes.float8_e3m4
    # host-side prep mirrors the sharding hint: replicate the normalized
    # input; give each shard its (normalized) weight slice
    xn = x / np.maximum(
        np.sqrt(np.einsum("bd,bd->b", x, x, dtype=np.float64)), EPS
    )[:, None].astype(np.float32)
    # xnt[bi, p, kd*128 + b'] = xn[bi*128 + b', kd*128 + p]
    xnt = xn.astype(bf).reshape(NB, 128, KD, 128).transpose(0, 3, 2, 1).reshape(
        NB, 128, KD * 128
    )
    xrows = np.ascontiguousarray(xnt).view(np.uint8).view(e3)  # [NB, 128, 1024]

    wnorm = np.maximum(
        np.sqrt(np.einsum("cd,cd->c", w, w, dtype=np.float64)), EPS
    ).astype(np.float32)
    in_maps = []
    for k in range(N_CORES):
        wk = w[k * CL : (k + 1) * CL] * (
            W_PRESCALE / wnorm[k * CL : (k + 1) * CL, None]
        )
        w8 = np.clip(wk, -15.5, 15.5).astype(e3)
        wt = np.zeros((NROWS, 2048), dtype=e3)
        wt[0:128, :1024] = xrows[0]
        wt[0:128, 1024:] = _wrows(w8[0:256])
        wt[128:256, :1024] = xrows[1]
        wt[128:256, 1024:] = _wrows(w8[256:512])
        wt[256:384, :1024] = xrows[2]
        wt[256:384, 1024:] = xrows[3]
        for i in range(2, 25):
            c0 = c0_of(i)
            wt[128 * (i + 1) : 128 * (i + 2), :] = _wrows(w8[c0 : c0 + 512])
        wt[128 * 26 :, :848] = _wrows(w8[12288:12500])
        in_maps.append({"wt": wt})
    return in_maps


def c0_of(i):
    return sum(CHUNKS[:i])


def kernel(input, weight, label):
    from concourse.bass_utils import run_bass_kernel_spmd

    nc = _build()
    x = np.ascontiguousarray(np.asarray(input, dtype=np.float32))
    w = np.ascontiguousarray(np.asarray(weight, dtype=np.float32))
    res = run_bass_kernel_spmd(nc, _in_maps(x, w), core_ids=list(range(N_CORES)))
    out = np.concatenate(
        [res.results[k]["out"][:, :CL] for k in range(N_CORES)], axis=1
    ).astype(np.float32)

    # ArcFace margin on the label column of each row (device emitted s*cos)
    rows = np.arange(B)
    cols = np.asarray(label).astype(np.int64)
    cos = out[rows, cols].astype(np.float64) / S_SCALE
    sine = np.sqrt(np.maximum(0.0, 1.0 - cos * cos))
    phi = cos * COS_M - sine * SIN_M
    phi = np.where(cos > TH, phi, cos - MM)
    out[rows, cols] = (phi * S_SCALE).astype(np.float32)
    return out
